# revision 6
# baseline (speedup 1.0000x reference)
"""Compressed Interaction Network (CIN) forward on 8 Trainium2 NeuronCores.

Math (per batch item, m=32 fields, d=64 embed, H=256 hidden):
    x0 = x[i]                          # (m, d)
    h  = x0
    layer l in 0..2:
        z = outer(x0, h) over d        # (m*n, d), z[(a,b),:] = x0[a,:]*h[b,:]
        y = relu(W_l^T z + b_l)        # (H, d)
        xcur, h = split_half(y) (layers 0,1); xcur = h = y (layer 2)
    f = concat(xcur_0, xcur_1, xcur_2) # (512, d)
    out[i] = sum_d(f) @ fc_W + fc_b    # scalar

Mapping: batch 1024 -> 8 cores x 128 items, 16 groups of 8 items per core.

The three layers of a group are software-pipelined across emission rounds so
the PE never stalls on the ScalarE h-drain -> VectorE z-production latency:
round r runs L0(r), L1(r-1), L2(r-2), fc(r-3) back to back on the PE while
VectorE builds the z tiles one round ahead of their consumption.
 - Layer 0 exploits z-symmetry (z[(a,b)] == z[(b,a)] since h == x0): W0 is
   host-folded onto the 528 unordered pairs (padded to 5 k-chunks of 128),
   cutting layer-0 matmul and vector work by 3/8. The pair operands come
   from two host-gathered tensors xu/xv so each k-chunk is ONE
   128-partition vector multiply.
 - Layers 1/2 build z by broadcasting h along a 4-field axis against a
   DMA-broadcast replica of x (Bg).
 - Conv matmuls on PE: stationary W chunks [128, 128] fp16, moving z
   [128, 512] (8 items x 64 d), accumulated over k-chunks in fp32 PSUM.
 - Bias+ReLU fused into the PSUM->SBUF drain on ScalarE; the relu'd xcur
   chunks go back through the PE against fc_W chunks, accumulating the
   per-(item,d) FC partial dot in a [1, 512] PSUM bank; one small VectorE
   reduce per group finishes the sum over d.
 - Big DMAs (Bg, W1, W2) issue on the Activation hwdge queue so the
   latency-critical per-group operands on the SP queue aren't stuck
   behind them during pipeline fill.
"""

import numpy as np

import concourse.bass as bass
import concourse.tile as tile
from concourse import mybir
from concourse.bass_utils import run_bass_kernel_spmd

N_CORES = 8
B_TOTAL = 1024
B_CORE = B_TOTAL // N_CORES  # 128
M = 32  # num fields
D = 64  # embed dim
H = 256  # conv output channels
GROUP = 8  # items per group (512 moving columns)
NG = B_CORE // GROUP  # 16 groups
MD = M * D  # 2048, elements per item row
KC0 = 5  # layer-0 k-chunks after symmetric folding (528 pairs -> 5*128)

F16 = mybir.dt.float16
F32 = mybir.dt.float32
RELU = mybir.ActivationFunctionType.Relu
IDENT = mybir.ActivationFunctionType.Identity
AXX = mybir.AxisListType.X
ADD = mybir.AluOpType.add


def build():
    nc = bass.Bass()
    xh = nc.declare_dram_parameter("xh", [B_CORE, M, D], F16, isOutput=False)
    # symmetric-pair operands: xu[i, c, p, d] = x[i, A[128c+p], d], ditto xv/B
    xu = nc.declare_dram_parameter("xu", [B_CORE, KC0, 128, D], F16, isOutput=False)
    xv = nc.declare_dram_parameter("xv", [B_CORE, KC0, 128, D], F16, isOutput=False)
    w0 = nc.declare_dram_parameter("w0", [KC0, 128, H], F16, isOutput=False)
    w1 = nc.declare_dram_parameter("w1", [32, 128, H], F16, isOutput=False)
    w2 = nc.declare_dram_parameter("w2", [32, 128, H], F16, isOutput=False)
    bia = nc.declare_dram_parameter("bia", [128, 3, 2], F32, isOutput=False)
    fcw = nc.declare_dram_parameter("fcw", [128, 4], F16, isOutput=False)
    fcb = nc.declare_dram_parameter("fcb", [1, 1], F32, isOutput=False)
    out = nc.declare_dram_parameter("out", [B_CORE, 1], F32, isOutput=True)

    with tile.TileContext(nc) as tc:
        with (
            tc.tile_pool(name="consts", bufs=1) as consts,
            tc.tile_pool(name="bpool", bufs=3) as bpool,
            tc.tile_pool(name="upool", bufs=2) as upool,
            tc.tile_pool(name="z0pool", bufs=8) as z0pool,
            tc.tile_pool(name="ztpool", bufs=3) as ztpool,
            tc.tile_pool(name="hpool", bufs=3) as hpool,
            tc.tile_pool(name="rxpool", bufs=5) as rxpool,
            tc.tile_pool(name="ppool", bufs=6, space="PSUM") as ppool,
            tc.tile_pool(name="fcp", bufs=2, space="PSUM") as fcp,
        ):
            # --- constants; w1/w2 go on the Activation queue in round 0 ---
            w0_sb = consts.tile([128, KC0, H], F16, tag="w0")
            nc.sync.dma_start(w0_sb[:], w0[:].rearrange("c k o -> k c o"))
            bia_sb = consts.tile([128, 3, 2], F32, tag="bia")
            nc.sync.dma_start(bia_sb[:], bia[:])
            fcw_sb = consts.tile([128, 4], F16, tag="fcw")
            nc.sync.dma_start(fcw_sb[:], fcw[:])
            fcb_sb = consts.tile([1, 1], F32, tag="fcb")
            nc.sync.dma_start(fcb_sb[:], fcb[:])
            w1_sb = consts.tile([128, 32, H], F16, tag="w1")
            w2_sb = consts.tile([128, 32, H], F16, tag="w2")

            # per-item FC dot results, [1, item]
            osb = consts.tile([1, B_CORE], F32, tag="osb")

            # pipeline state carried between rounds
            Ug = {}
            Vg = {}
            Bg = {}
            z0t = {}
            h1t = {}
            h2t = {}
            rx0t = {}
            rx1t = {}
            rx2t = {}
            ps0t = {}
            ps1t = {}
            ps2t = {}
            fc_done = set()

            def dma_group_uv(g):
                for name, dram, store in (("U", xu, Ug), ("V", xv, Vg)):
                    t = upool.tile([128, GROUP, KC0, D], F16, tag=name, name=name)
                    src = bass.AP(
                        tensor=dram,
                        offset=g * GROUP * KC0 * 128 * D,
                        ap=[
                            [D, 128],
                            [KC0 * 128 * D, GROUP],
                            [128 * D, KC0],
                            [1, D],
                        ],
                    )
                    nc.sync.dma_start(t[:], src)
                    store[g] = t

            def dma_group_b(g):
                Bg[g] = bpool.tile([128, GROUP, M, D], F16, tag="B", name="Bg")
                src = bass.AP(
                    tensor=xh,
                    offset=g * GROUP * MD,
                    ap=[[0, 128], [MD, GROUP], [1, MD]],
                )
                nc.scalar.dma_start(Bg[g][:], src)

            def vec_z0(g):
                z0t[g] = [
                    z0pool.tile([128, GROUP, D], F16, tag="z0", name="z0")
                    for _ in range(KC0)
                ]
                for c in range(KC0):
                    nc.vector.tensor_mul(
                        z0t[g][c][:], Ug[g][:, :, c, :], Vg[g][:, :, c, :]
                    )

            # ---------- preamble: group 0's z0 ----------
            dma_group_uv(0)
            vec_z0(0)

            for r in range(NG + 2):
                # 1. prefetch DMAs
                if r + 1 < NG:
                    dma_group_uv(r + 1)
                if r < NG:
                    dma_group_b(r)
                if r == 0:
                    nc.scalar.dma_start(
                        w1_sb[:], w1[:].rearrange("c k o -> k c o")
                    )
                    nc.scalar.dma_start(
                        w2_sb[:], w2[:].rearrange("c k o -> k c o")
                    )

                # 2. vector: zt for L1(r-1)
                if 0 <= r - 1 < NG:
                    g = r - 1
                    zl1 = [
                        ztpool.tile([128, GROUP, 4, D], F16, tag="z1", name="zl1")
                        for _ in range(8)
                    ]
                    for mb in range(8):
                        nc.vector.tensor_mul(
                            zl1[mb][:],
                            h1t[g][:, :, None, :].to_broadcast((128, GROUP, 4, D)),
                            Bg[g][:, :, 4 * mb : 4 * mb + 4, :],
                        )

                # 3. PE: L0(r)
                if r < NG:
                    ps0t[r] = [
                        ppool.tile([128, GROUP * D], F32, tag="yps", name="ps0")
                        for _ in range(2)
                    ]
                    for c in range(KC0):
                        for oc in range(2):
                            nc.tensor.matmul(
                                ps0t[r][oc][:],
                                w0_sb[:, c, oc * 128 : (oc + 1) * 128],
                                z0t[r][c][:],
                                start=(c == 0),
                                stop=(c == KC0 - 1),
                            )
                    del z0t[r]

                # 4. scalar: drain L0(r)
                if r < NG:
                    h1t[r] = hpool.tile([128, GROUP, D], F16, tag="h1", name="h1")
                    nc.scalar.activation(
                        h1t[r][:], ps0t[r][1][:], RELU, bias=bia_sb[:, 0, 1:2]
                    )
                    rx0t[r] = rxpool.tile([128, GROUP * D], F16, tag="rx0", name="rx0")
                    nc.scalar.activation(
                        rx0t[r][:], ps0t[r][0][:], RELU, bias=bia_sb[:, 0, 0:1]
                    )
                    del ps0t[r]

                # 5. PE: L1(r-1)
                if 0 <= r - 1 < NG:
                    g = r - 1
                    ps1t[g] = [
                        ppool.tile([128, GROUP * D], F32, tag="yps", name="ps1")
                        for _ in range(2)
                    ]
                    for mb in range(8):
                        for mm in range(4):
                            m = 4 * mb + mm
                            for oc in range(2):
                                nc.tensor.matmul(
                                    ps1t[g][oc][:],
                                    w1_sb[:, m, oc * 128 : (oc + 1) * 128],
                                    zl1[mb][:, :, mm, :],
                                    start=(m == 0),
                                    stop=(m == 31),
                                )

                # 6. vector: z0(r+1)
                if r + 1 < NG:
                    vec_z0(r + 1)

                # 7. scalar: drain L1(r-1)
                if 0 <= r - 1 < NG:
                    g = r - 1
                    h2t[g] = hpool.tile([128, GROUP, D], F16, tag="h2", name="h2")
                    nc.scalar.activation(
                        h2t[g][:], ps1t[g][1][:], RELU, bias=bia_sb[:, 1, 1:2]
                    )
                    rx1t[g] = rxpool.tile([128, GROUP * D], F16, tag="rx1", name="rx1")
                    nc.scalar.activation(
                        rx1t[g][:], ps1t[g][0][:], RELU, bias=bia_sb[:, 1, 0:1]
                    )
                    del ps1t[g], h1t[g]

                # 8. vector: zt for L2(r-2)
                if 0 <= r - 2 < NG:
                    g = r - 2
                    zl2 = [
                        ztpool.tile([128, GROUP, 4, D], F16, tag="z2", name="zl2")
                        for _ in range(8)
                    ]
                    for mb in range(8):
                        nc.vector.tensor_mul(
                            zl2[mb][:],
                            h2t[g][:, :, None, :].to_broadcast((128, GROUP, 4, D)),
                            Bg[g][:, :, 4 * mb : 4 * mb + 4, :],
                        )

                # 9. PE: L2(r-2)
                if 0 <= r - 2 < NG:
                    g = r - 2
                    ps2t[g] = [
                        ppool.tile([128, GROUP * D], F32, tag="yps", name="ps2")
                        for _ in range(2)
                    ]
                    for mb in range(8):
                        for mm in range(4):
                            m = 4 * mb + mm
                            for oc in range(2):
                                nc.tensor.matmul(
                                    ps2t[g][oc][:],
                                    w2_sb[:, m, oc * 128 : (oc + 1) * 128],
                                    zl2[mb][:, :, mm, :],
                                    start=(m == 0),
                                    stop=(m == 31),
                                )

                # 10. scalar: drain L2(r-2)
                if 0 <= r - 2 < NG:
                    g = r - 2
                    rx2t[g] = [
                        rxpool.tile([128, GROUP * D], F16, tag="rx2", name="rx2")
                        for _ in range(2)
                    ]
                    for oc in range(2):
                        nc.scalar.activation(
                            rx2t[g][oc][:],
                            ps2t[g][oc][:],
                            RELU,
                            bias=bia_sb[:, 2, oc : oc + 1],
                        )
                    del ps2t[g], h2t[g], Bg[g]

                # 11. PE + vector: FC dot (group r-3; last group pulled in a
                # round early so the pipeline drains one round sooner)
                fc_groups = [r - 3]
                if r - 2 == NG - 1:
                    fc_groups.append(r - 2)
                for g in fc_groups:
                    if not (0 <= g < NG) or g in fc_done:
                        continue
                    fc_done.add(g)
                    fc_ps = fcp.tile([1, GROUP * D], F32, tag="fc", name="fc")
                    rxs = [rx0t[g], rx1t[g], rx2t[g][0], rx2t[g][1]]
                    for c in range(4):
                        nc.tensor.matmul(
                            fc_ps[:],
                            fcw_sb[:, c : c + 1],
                            rxs[c][:],
                            start=(c == 0),
                            stop=(c == 3),
                        )
                    nc.vector.tensor_reduce(
                        osb[0:1, g * GROUP : (g + 1) * GROUP],
                        fc_ps[:].rearrange("p (i d) -> p i d", i=GROUP),
                        axis=AXX,
                        op=ADD,
                    )
                    del rx0t[g], rx1t[g], rx2t[g]

            # ---------- finalize: add fc bias, write out ----------
            osb2 = consts.tile([1, B_CORE], F32, tag="osb2")
            nc.scalar.activation(osb2[:], osb[:], IDENT, bias=fcb_sb[0:1, 0:1])
            nc.sync.dma_start(out[:], osb2[:])

    _legalize_waits(nc)
    return nc


def _legalize_waits(nc, max_waits=1):
    """walrus codegen allows at most 2 semaphore waits per instruction; spill
    the excess onto NoOps injected just before the offender on the same
    engine (same-engine FIFO makes this ordering-equivalent)."""
    for bb in nc.main_func.blocks:
        insts = bb.instructions
        new_list = []
        changed = False
        for ins in insts:
            si = ins.sync_info
            if si is not None and si.on_wait and len(si.on_wait) > max_waits:
                waits = list(si.on_wait)
                extra, keep = waits[:-max_waits], waits[-max_waits:]
                k = 0
                while k < len(extra):
                    chunk = extra[k : k + max_waits]
                    nop = mybir.InstNoOp(name=f"{ins.name}-w{k}", ins=[], outs=[])
                    nop.engine = ins.engine
                    nop.sync_info = mybir.SyncInfo(on_wait=chunk, on_update=[])
                    new_list.append(nop)
                    k += max_waits
                ins.sync_info = mybir.SyncInfo(
                    on_wait=keep,
                    on_update=list(si.on_update) if si.on_update else [],
                )
                changed = True
            new_list.append(ins)
        if changed:
            if hasattr(bb, "set_instructions"):
                bb.set_instructions(new_list)
            else:
                insts.clear()
                insts.extend(new_list)
                if len(bb.instructions) != len(new_list):
                    bb.instructions = new_list


def _sym_pairs():
    """Unordered field pairs (a<=b), padded to KC0*128 with zero rows."""
    pairs = [(a, b) for a in range(M) for b in range(a, M)]
    n = len(pairs)  # 528
    pad = KC0 * 128 - n
    A = np.array([p[0] for p in pairs] + [0] * pad)
    B = np.array([p[1] for p in pairs] + [0] * pad)
    mask = np.arange(KC0 * 128) < n
    return A, B, mask


def prep_inputs(x, W0, b0, W1, b1, W2, b2, fc_W, fc_b):
    """Host-side reshape/cast into the per-core input maps."""
    xh = np.ascontiguousarray(x.astype(np.float16))
    A, B, mask = _sym_pairs()
    # folded layer-0 weights over unordered pairs
    Wf = W0[A * M + B] + np.where((A != B)[:, None], W0[B * M + A], 0.0)
    Wf = np.where(mask[:, None], Wf, 0.0)
    xu = np.ascontiguousarray(xh[:, A, :].reshape(B_TOTAL, KC0, 128, D))
    xv = np.ascontiguousarray(xh[:, B, :].reshape(B_TOTAL, KC0, 128, D))
    w0 = np.ascontiguousarray(Wf.astype(np.float16).reshape(KC0, 128, H))
    w1 = np.ascontiguousarray(W1.astype(np.float16).reshape(32, 128, H))
    w2 = np.ascontiguousarray(W2.astype(np.float16).reshape(32, 128, H))
    bia = np.ascontiguousarray(
        np.stack([b0, b1, b2]).reshape(3, 2, 128).transpose(2, 0, 1).astype(np.float32)
    )
    fcw = np.ascontiguousarray(fc_W.reshape(4, 128).T.astype(np.float16))
    fcb = np.ascontiguousarray(fc_b.reshape(1, 1).astype(np.float32))
    shared = {"w0": w0, "w1": w1, "w2": w2, "bia": bia, "fcw": fcw, "fcb": fcb}
    return [
        {
            "xh": xh[i * B_CORE : (i + 1) * B_CORE],
            "xu": xu[i * B_CORE : (i + 1) * B_CORE],
            "xv": xv[i * B_CORE : (i + 1) * B_CORE],
            **shared,
        }
        for i in range(N_CORES)
    ]


_NC = None


def _get_nc():
    global _NC
    if _NC is None:
        _NC = build()
    return _NC


def kernel(**inputs):
    in_maps = prep_inputs(**inputs)
    res = run_bass_kernel_spmd(_get_nc(), in_maps, list(range(N_CORES)))
    return np.ascontiguousarray(
        np.concatenate([r["out"] for r in res.results], axis=0).astype(np.float32)
    )


# revision 11
# speedup vs baseline: 1.0486x; 1.0486x over previous
"""Compressed Interaction Network (CIN) forward on 8 Trainium2 NeuronCores.

Math (per batch item, m=32 fields, d=64 embed, H=256 hidden):
    x0 = x[i]                          # (m, d)
    h  = x0
    layer l in 0..2:
        z = outer(x0, h) over d        # (m*n, d), z[(a,b),:] = x0[a,:]*h[b,:]
        y = relu(W_l^T z + b_l)        # (H, d)
        xcur, h = split_half(y) (layers 0,1); xcur = h = y (layer 2)
    f = concat(xcur_0, xcur_1, xcur_2) # (512, d)
    out[i] = sum_d(f) @ fc_W + fc_b    # scalar

Mapping: batch 1024 -> 8 cores x 128 items, 16 groups of 8 items per core.

The three layers of a group are software-pipelined across emission rounds so
the PE never stalls on the ScalarE h-drain -> VectorE z-production latency:
round r runs L0(r), L1(r-1), L2(r-2), fc(r-3) back to back on the PE while
VectorE builds the z tiles one round ahead of their consumption.
 - Layer 0 exploits z-symmetry (z[(a,b)] == z[(b,a)] since h == x0): W0 is
   host-folded onto the 528 unordered pairs (padded to 5 k-chunks of 128),
   cutting layer-0 matmul and vector work by 3/8. The pair operands come
   from two host-gathered tensors xu/xv so each k-chunk is ONE
   128-partition vector multiply.
 - Layers 1/2 build z by broadcasting h along a 4-field axis against a
   DMA-broadcast replica of x (Bg).
 - Conv matmuls on PE: stationary W chunks [128, 128] fp16, moving z
   [128, 512] (8 items x 64 d), accumulated over k-chunks in fp32 PSUM.
 - Bias+ReLU fused into the PSUM->SBUF drain on ScalarE; the relu'd xcur
   chunks go back through the PE against fc_W chunks, accumulating the
   per-(item,d) FC partial dot in a [1, 512] PSUM bank; one small VectorE
   reduce per group finishes the sum over d.
 - Big DMAs (Bg, W1, W2) issue on the Activation hwdge queue so the
   latency-critical per-group operands on the SP queue aren't stuck
   behind them during pipeline fill.
"""

import numpy as np

import concourse.bass as bass
import concourse.tile as tile
from concourse import mybir
from concourse.bass_utils import run_bass_kernel_spmd

N_CORES = 8
B_TOTAL = 1024
B_CORE = B_TOTAL // N_CORES  # 128
M = 32  # num fields
D = 64  # embed dim
H = 256  # conv output channels
GROUP = 8  # items per group (512 moving columns)
NG = B_CORE // GROUP  # 16 groups
MD = M * D  # 2048, elements per item row
KC0 = 5  # layer-0 k-chunks after symmetric folding (528 pairs -> 5*128)

F16 = mybir.dt.float16
F32 = mybir.dt.float32
RELU = mybir.ActivationFunctionType.Relu
IDENT = mybir.ActivationFunctionType.Identity
AXX = mybir.AxisListType.X
ADD = mybir.AluOpType.add


def build():
    nc = bass.Bass()
    xh = nc.declare_dram_parameter("xh", [B_CORE, M, D], F16, isOutput=False)
    # symmetric-pair operands, partition-major so each partition's group
    # slice is one contiguous 5 KB DMA read:
    #   xu[p, i, c, d] = x[i, A[128c + p], d], ditto xv/B
    xu = nc.declare_dram_parameter("xu", [128, B_CORE, KC0, D], F16, isOutput=False)
    xv = nc.declare_dram_parameter("xv", [128, B_CORE, KC0, D], F16, isOutput=False)
    w0 = nc.declare_dram_parameter("w0", [KC0, 128, H], F16, isOutput=False)
    w1 = nc.declare_dram_parameter("w1", [32, 128, H], F16, isOutput=False)
    w2 = nc.declare_dram_parameter("w2", [32, 128, H], F16, isOutput=False)
    bia = nc.declare_dram_parameter("bia", [128, 3, 2], F32, isOutput=False)
    fcw = nc.declare_dram_parameter("fcw", [128, 4], F16, isOutput=False)
    fcb = nc.declare_dram_parameter("fcb", [1, 1], F32, isOutput=False)
    out = nc.declare_dram_parameter("out", [B_CORE, 1], F32, isOutput=True)

    with tile.TileContext(nc) as tc:
        with (
            tc.tile_pool(name="consts", bufs=1) as consts,
            tc.tile_pool(name="bpool", bufs=3) as bpool,
            tc.tile_pool(name="upool", bufs=2) as upool,
            tc.tile_pool(name="z0pool", bufs=8) as z0pool,
            tc.tile_pool(name="ztpool", bufs=3) as ztpool,
            tc.tile_pool(name="hpool", bufs=3) as hpool,
            tc.tile_pool(name="rxpool", bufs=5) as rxpool,
            tc.tile_pool(name="ppool", bufs=6, space="PSUM") as ppool,
            tc.tile_pool(name="fcp", bufs=2, space="PSUM") as fcp,
        ):
            # --- constants; w1/w2 go on the Activation queue in round 0 ---
            w0_sb = consts.tile([128, KC0, H], F16, tag="w0")
            nc.sync.dma_start(w0_sb[:], w0[:].rearrange("c k o -> k c o"))
            bia_sb = consts.tile([128, 3, 2], F32, tag="bia")
            nc.sync.dma_start(bia_sb[:], bia[:])
            fcw_sb = consts.tile([128, 4], F16, tag="fcw")
            nc.sync.dma_start(fcw_sb[:], fcw[:])
            fcb_sb = consts.tile([1, 1], F32, tag="fcb")
            nc.sync.dma_start(fcb_sb[:], fcb[:])
            w1_sb = consts.tile([128, 32, H], F16, tag="w1")
            w2_sb = consts.tile([128, 32, H], F16, tag="w2")

            # per-item FC dot results, [1, item]
            osb = consts.tile([1, B_CORE], F32, tag="osb")

            # pipeline state carried between rounds
            Ug = {}
            Vg = {}
            Bg = {}
            z0t = {}
            h1t = {}
            h2t = {}
            rx0t = {}
            rx1t = {}
            rx2t = {}
            ps0t = {}
            ps1t = {}
            ps2t = {}
            fc_done = set()

            def dma_group_uv(g):
                for name, dram, store in (("U", xu, Ug), ("V", xv, Vg)):
                    t = upool.tile([128, GROUP, KC0, D], F16, tag=name, name=name)
                    src = bass.AP(
                        tensor=dram,
                        offset=g * GROUP * KC0 * D,
                        ap=[
                            [B_CORE * KC0 * D, 128],
                            [1, GROUP * KC0 * D],
                        ],
                    )
                    nc.sync.dma_start(t[:], src)
                    store[g] = t

            def dma_group_b(g):
                Bg[g] = bpool.tile([128, GROUP, M, D], F16, tag="B", name="Bg")
                src = bass.AP(
                    tensor=xh,
                    offset=g * GROUP * MD,
                    ap=[[0, 128], [MD, GROUP], [1, MD]],
                )
                nc.scalar.dma_start(Bg[g][:], src)

            def vec_z0(g):
                z0t[g] = [
                    z0pool.tile([128, GROUP, D], F16, tag="z0", name="z0")
                    for _ in range(KC0)
                ]
                for c in range(KC0):
                    nc.vector.tensor_mul(
                        z0t[g][c][:], Ug[g][:, :, c, :], Vg[g][:, :, c, :]
                    )

            # ---------- preamble: group 0's z0 ----------
            dma_group_uv(0)
            vec_z0(0)

            for r in range(NG + 2):
                # 1. prefetch DMAs
                if r + 1 < NG:
                    dma_group_uv(r + 1)
                if r < NG:
                    dma_group_b(r)
                if r == 0:
                    nc.scalar.dma_start(
                        w1_sb[:], w1[:].rearrange("c k o -> k c o")
                    )
                if r == 1:
                    nc.scalar.dma_start(
                        w2_sb[:], w2[:].rearrange("c k o -> k c o")
                    )

                # 2. vector: zt for L1(r-1)
                if 0 <= r - 1 < NG:
                    g = r - 1
                    zl1 = [
                        ztpool.tile([128, GROUP, 4, D], F16, tag="z1", name="zl1")
                        for _ in range(8)
                    ]
                    for mb in range(8):
                        nc.vector.tensor_mul(
                            zl1[mb][:],
                            h1t[g][:, :, None, :].to_broadcast((128, GROUP, 4, D)),
                            Bg[g][:, :, 4 * mb : 4 * mb + 4, :],
                        )

                # 3. PE: L0(r)
                if r < NG:
                    ps0t[r] = [
                        ppool.tile([128, GROUP * D], F32, tag="yps", name="ps0")
                        for _ in range(2)
                    ]
                    for c in range(KC0):
                        for oc in range(2):
                            nc.tensor.matmul(
                                ps0t[r][oc][:],
                                w0_sb[:, c, oc * 128 : (oc + 1) * 128],
                                z0t[r][c][:],
                                start=(c == 0),
                                stop=(c == KC0 - 1),
                            )
                    del z0t[r]

                # 4. scalar: drain L0(r)
                if r < NG:
                    h1t[r] = hpool.tile([128, GROUP, D], F16, tag="h1", name="h1")
                    nc.scalar.activation(
                        h1t[r][:], ps0t[r][1][:], RELU, bias=bia_sb[:, 0, 1:2]
                    )
                    rx0t[r] = rxpool.tile([128, GROUP * D], F16, tag="rx0", name="rx0")
                    nc.scalar.activation(
                        rx0t[r][:], ps0t[r][0][:], RELU, bias=bia_sb[:, 0, 0:1]
                    )
                    del ps0t[r]

                # 5. PE: L1(r-1)
                if 0 <= r - 1 < NG:
                    g = r - 1
                    ps1t[g] = [
                        ppool.tile([128, GROUP * D], F32, tag="yps", name="ps1")
                        for _ in range(2)
                    ]
                    for mb in range(8):
                        for mm in range(4):
                            m = 4 * mb + mm
                            for oc in range(2):
                                nc.tensor.matmul(
                                    ps1t[g][oc][:],
                                    w1_sb[:, m, oc * 128 : (oc + 1) * 128],
                                    zl1[mb][:, :, mm, :],
                                    start=(m == 0),
                                    stop=(m == 31),
                                )

                # 6. vector: z0(r+1)
                if r + 1 < NG:
                    vec_z0(r + 1)

                # 7. scalar: drain L1(r-1)
                if 0 <= r - 1 < NG:
                    g = r - 1
                    h2t[g] = hpool.tile([128, GROUP, D], F16, tag="h2", name="h2")
                    nc.scalar.activation(
                        h2t[g][:], ps1t[g][1][:], RELU, bias=bia_sb[:, 1, 1:2]
                    )
                    rx1t[g] = rxpool.tile([128, GROUP * D], F16, tag="rx1", name="rx1")
                    nc.scalar.activation(
                        rx1t[g][:], ps1t[g][0][:], RELU, bias=bia_sb[:, 1, 0:1]
                    )
                    del ps1t[g], h1t[g]

                # 8. vector: zt for L2(r-2)
                if 0 <= r - 2 < NG:
                    g = r - 2
                    zl2 = [
                        ztpool.tile([128, GROUP, 4, D], F16, tag="z2", name="zl2")
                        for _ in range(8)
                    ]
                    for mb in range(8):
                        nc.vector.tensor_mul(
                            zl2[mb][:],
                            h2t[g][:, :, None, :].to_broadcast((128, GROUP, 4, D)),
                            Bg[g][:, :, 4 * mb : 4 * mb + 4, :],
                        )

                # 9. PE: L2(r-2)
                if 0 <= r - 2 < NG:
                    g = r - 2
                    ps2t[g] = [
                        ppool.tile([128, GROUP * D], F32, tag="yps", name="ps2")
                        for _ in range(2)
                    ]
                    for mb in range(8):
                        for mm in range(4):
                            m = 4 * mb + mm
                            for oc in range(2):
                                nc.tensor.matmul(
                                    ps2t[g][oc][:],
                                    w2_sb[:, m, oc * 128 : (oc + 1) * 128],
                                    zl2[mb][:, :, mm, :],
                                    start=(m == 0),
                                    stop=(m == 31),
                                )

                # 10. scalar: drain L2(r-2)
                if 0 <= r - 2 < NG:
                    g = r - 2
                    rx2t[g] = [
                        rxpool.tile([128, GROUP * D], F16, tag="rx2", name="rx2")
                        for _ in range(2)
                    ]
                    for oc in range(2):
                        nc.scalar.activation(
                            rx2t[g][oc][:],
                            ps2t[g][oc][:],
                            RELU,
                            bias=bia_sb[:, 2, oc : oc + 1],
                        )
                    del ps2t[g], h2t[g], Bg[g]

                # 11. PE + vector: FC dot (group r-3; last group pulled in a
                # round early so the pipeline drains one round sooner)
                fc_groups = [r - 3]
                if r - 2 == NG - 1:
                    fc_groups.append(r - 2)
                for g in fc_groups:
                    if not (0 <= g < NG) or g in fc_done:
                        continue
                    fc_done.add(g)
                    fc_ps = fcp.tile([1, GROUP * D], F32, tag="fc", name="fc")
                    rxs = [rx0t[g], rx1t[g], rx2t[g][0], rx2t[g][1]]
                    for c in range(4):
                        nc.tensor.matmul(
                            fc_ps[:],
                            fcw_sb[:, c : c + 1],
                            rxs[c][:],
                            start=(c == 0),
                            stop=(c == 3),
                        )
                    nc.vector.tensor_reduce(
                        osb[0:1, g * GROUP : (g + 1) * GROUP],
                        fc_ps[:].rearrange("p (i d) -> p i d", i=GROUP),
                        axis=AXX,
                        op=ADD,
                    )
                    del rx0t[g], rx1t[g], rx2t[g]

            # ---------- finalize: add fc bias, write out ----------
            osb2 = consts.tile([1, B_CORE], F32, tag="osb2")
            nc.scalar.activation(osb2[:], osb[:], IDENT, bias=fcb_sb[0:1, 0:1])
            nc.sync.dma_start(out[:], osb2[:])

    _legalize_waits(nc)
    return nc


def _legalize_waits(nc, max_waits=1):
    """walrus codegen allows at most 2 semaphore waits per instruction; spill
    the excess onto NoOps injected just before the offender on the same
    engine (same-engine FIFO makes this ordering-equivalent)."""
    for bb in nc.main_func.blocks:
        insts = bb.instructions
        new_list = []
        changed = False
        for ins in insts:
            si = ins.sync_info
            if si is not None and si.on_wait and len(si.on_wait) > max_waits:
                waits = list(si.on_wait)
                extra, keep = waits[:-max_waits], waits[-max_waits:]
                k = 0
                while k < len(extra):
                    chunk = extra[k : k + max_waits]
                    nop = mybir.InstNoOp(name=f"{ins.name}-w{k}", ins=[], outs=[])
                    nop.engine = ins.engine
                    nop.sync_info = mybir.SyncInfo(on_wait=chunk, on_update=[])
                    new_list.append(nop)
                    k += max_waits
                ins.sync_info = mybir.SyncInfo(
                    on_wait=keep,
                    on_update=list(si.on_update) if si.on_update else [],
                )
                changed = True
            new_list.append(ins)
        if changed:
            if hasattr(bb, "set_instructions"):
                bb.set_instructions(new_list)
            else:
                insts.clear()
                insts.extend(new_list)
                if len(bb.instructions) != len(new_list):
                    bb.instructions = new_list


def _sym_pairs():
    """Unordered field pairs (a<=b), padded to KC0*128 with zero rows."""
    pairs = [(a, b) for a in range(M) for b in range(a, M)]
    n = len(pairs)  # 528
    pad = KC0 * 128 - n
    A = np.array([p[0] for p in pairs] + [0] * pad)
    B = np.array([p[1] for p in pairs] + [0] * pad)
    mask = np.arange(KC0 * 128) < n
    return A, B, mask


def prep_inputs(x, W0, b0, W1, b1, W2, b2, fc_W, fc_b):
    """Host-side reshape/cast into the per-core input maps."""
    xh = np.ascontiguousarray(x.astype(np.float16))
    A, B, mask = _sym_pairs()
    # folded layer-0 weights over unordered pairs
    Wf = W0[A * M + B] + np.where((A != B)[:, None], W0[B * M + A], 0.0)
    Wf = np.where(mask[:, None], Wf, 0.0)
    # [p, i, c, d] layout: per-partition group slices are contiguous
    xu = np.ascontiguousarray(
        xh[:, A, :].reshape(B_TOTAL, KC0, 128, D).transpose(2, 0, 1, 3)
    )
    xv = np.ascontiguousarray(
        xh[:, B, :].reshape(B_TOTAL, KC0, 128, D).transpose(2, 0, 1, 3)
    )
    w0 = np.ascontiguousarray(Wf.astype(np.float16).reshape(KC0, 128, H))
    w1 = np.ascontiguousarray(W1.astype(np.float16).reshape(32, 128, H))
    w2 = np.ascontiguousarray(W2.astype(np.float16).reshape(32, 128, H))
    bia = np.ascontiguousarray(
        np.stack([b0, b1, b2]).reshape(3, 2, 128).transpose(2, 0, 1).astype(np.float32)
    )
    fcw = np.ascontiguousarray(fc_W.reshape(4, 128).T.astype(np.float16))
    fcb = np.ascontiguousarray(fc_b.reshape(1, 1).astype(np.float32))
    shared = {"w0": w0, "w1": w1, "w2": w2, "bia": bia, "fcw": fcw, "fcb": fcb}
    return [
        {
            "xh": xh[i * B_CORE : (i + 1) * B_CORE],
            "xu": np.ascontiguousarray(xu[:, i * B_CORE : (i + 1) * B_CORE]),
            "xv": np.ascontiguousarray(xv[:, i * B_CORE : (i + 1) * B_CORE]),
            **shared,
        }
        for i in range(N_CORES)
    ]


_NC = None


def _get_nc():
    global _NC
    if _NC is None:
        _NC = build()
    return _NC


def kernel(**inputs):
    in_maps = prep_inputs(**inputs)
    res = run_bass_kernel_spmd(_get_nc(), in_maps, list(range(N_CORES)))
    return np.ascontiguousarray(
        np.concatenate([r["out"] for r in res.results], axis=0).astype(np.float32)
    )


# revision 19
# speedup vs baseline: 1.0612x; 1.0120x over previous
"""Compressed Interaction Network (CIN) forward on 8 Trainium2 NeuronCores.

Math (per batch item, m=32 fields, d=64 embed, H=256 hidden):
    x0 = x[i]                          # (m, d)
    h  = x0
    layer l in 0..2:
        z = outer(x0, h) over d        # (m*n, d), z[(a,b),:] = x0[a,:]*h[b,:]
        y = relu(W_l^T z + b_l)        # (H, d)
        xcur, h = split_half(y) (layers 0,1); xcur = h = y (layer 2)
    f = concat(xcur_0, xcur_1, xcur_2) # (512, d)
    out[i] = sum_d(f) @ fc_W + fc_b    # scalar

Mapping: batch 1024 -> 8 cores x 128 items, 16 groups of 8 items per core.

The three layers of a group are software-pipelined across emission rounds so
the PE never stalls on the ScalarE h-drain -> VectorE z-production latency:
round r runs L0(r), L1(r-1), L2(r-2), fc(r-3) back to back on the PE while
VectorE builds the z tiles one round ahead of their consumption.
 - Layer 0 exploits z-symmetry (z[(a,b)] == z[(b,a)] since h == x0): W0 is
   host-folded onto the 528 unordered pairs (padded to 5 k-chunks of 128),
   cutting layer-0 matmul and vector work by 3/8. The pair operands come
   from two host-gathered tensors xu/xv so each k-chunk is ONE
   128-partition vector multiply.
 - Layers 1/2 build z by broadcasting h along a 4-field axis against a
   DMA-broadcast replica of x (Bg).
 - Conv matmuls on PE: stationary W chunks [128, 128] fp16, moving z
   [128, 512] (8 items x 64 d), accumulated over k-chunks in fp32 PSUM.
 - Bias+ReLU fused into the PSUM->SBUF drain on ScalarE; the relu'd xcur
   chunks go back through the PE against fc_W chunks, accumulating the
   per-(item,d) FC partial dot in a [1, 512] PSUM bank; one small VectorE
   reduce per group finishes the sum over d.
 - Big DMAs (Bg, W1, W2) issue on the Activation hwdge queue so the
   latency-critical per-group operands on the SP queue aren't stuck
   behind them during pipeline fill.
"""

import numpy as np

import concourse.bass as bass
import concourse.tile as tile
from concourse import mybir
from concourse.bass_utils import run_bass_kernel_spmd

N_CORES = 8
B_TOTAL = 1024
B_CORE = B_TOTAL // N_CORES  # 128
M = 32  # num fields
D = 64  # embed dim
H = 256  # conv output channels
GROUP = 8  # items per group (512 moving columns)
NG = B_CORE // GROUP  # 16 groups
MD = M * D  # 2048, elements per item row
KC0 = 5  # layer-0 k-chunks after symmetric folding (528 pairs -> 5*128)

F16 = mybir.dt.float16
F32 = mybir.dt.float32
RELU = mybir.ActivationFunctionType.Relu
IDENT = mybir.ActivationFunctionType.Identity
AXX = mybir.AxisListType.X
ADD = mybir.AluOpType.add


def build():
    nc = bass.Bass()
    xh = nc.declare_dram_parameter("xh", [B_CORE, M, D], F16, isOutput=False)
    # symmetric-pair operands, partition-major so each partition's group
    # slice is one contiguous 5 KB DMA read:
    #   xu[p, i, c, d] = x[i, A[128c + p], d], ditto xv/B
    xu = nc.declare_dram_parameter("xu", [128, B_CORE, KC0, D], F16, isOutput=False)
    xv = nc.declare_dram_parameter("xv", [128, B_CORE, KC0, D], F16, isOutput=False)
    w0 = nc.declare_dram_parameter("w0", [KC0, 128, H], F16, isOutput=False)
    w1 = nc.declare_dram_parameter("w1", [32, 128, H], F16, isOutput=False)
    w2 = nc.declare_dram_parameter("w2", [32, 128, H], F16, isOutput=False)
    bia = nc.declare_dram_parameter("bia", [128, 3, 2], F32, isOutput=False)
    fcw = nc.declare_dram_parameter("fcw", [128, 4], F16, isOutput=False)
    fcb = nc.declare_dram_parameter("fcb", [1, 1], F32, isOutput=False)
    out = nc.declare_dram_parameter("out", [B_CORE, 1], F32, isOutput=True)

    with tile.TileContext(nc) as tc:
        with (
            tc.tile_pool(name="consts", bufs=1) as consts,
            tc.tile_pool(name="bpool", bufs=3) as bpool,
            tc.tile_pool(name="upool", bufs=2) as upool,
            tc.tile_pool(name="z0pool", bufs=8) as z0pool,
            tc.tile_pool(name="ztpool", bufs=3) as ztpool,
            tc.tile_pool(name="hpool", bufs=3) as hpool,
            tc.tile_pool(name="rxpool", bufs=5) as rxpool,
            tc.tile_pool(name="ppool", bufs=6, space="PSUM") as ppool,
            tc.tile_pool(name="fcp", bufs=2, space="PSUM") as fcp,
        ):
            # --- constants; w1/w2 go on the Activation queue in round 0 ---
            w0_sb = consts.tile([128, KC0, H], F16, tag="w0")
            nc.sync.dma_start(w0_sb[:], w0[:].rearrange("c k o -> k c o"))
            bia_sb = consts.tile([128, 3, 2], F32, tag="bia")
            nc.sync.dma_start(bia_sb[:], bia[:])
            fcw_sb = consts.tile([128, 4], F16, tag="fcw")
            nc.sync.dma_start(fcw_sb[:], fcw[:])
            fcb_sb = consts.tile([1, 1], F32, tag="fcb")
            nc.sync.dma_start(fcb_sb[:], fcb[:])
            w1_sb = consts.tile([128, 32, H], F16, tag="w1")
            w2_sb = consts.tile([128, 32, H], F16, tag="w2")

            # per-item FC dot results, [1, item]
            osb = consts.tile([1, B_CORE], F32, tag="osb")

            # pipeline state carried between rounds
            Ug = {}
            Vg = {}
            Bg = {}
            z0t = {}
            h1t = {}
            h2t = {}
            rx0t = {}
            rx1t = {}
            rx2t = {}
            ps0t = {}
            ps1t = {}
            ps2t = {}
            fc_done = set()

            def dma_group_uv(g):
                for name, dram, store in (("U", xu, Ug), ("V", xv, Vg)):
                    t = upool.tile([128, GROUP, KC0, D], F16, tag=name, name=name)
                    src = bass.AP(
                        tensor=dram,
                        offset=g * GROUP * KC0 * D,
                        ap=[
                            [B_CORE * KC0 * D, 128],
                            [1, GROUP * KC0 * D],
                        ],
                    )
                    nc.sync.dma_start(t[:], src)
                    store[g] = t

            def dma_group_b(g, split=False):
                Bg[g] = bpool.tile([128, GROUP, M, D], F16, tag="B", name="Bg")
                if split:
                    # halves on both hwdge queues so neither serializes fill
                    for eng, mlo in ((nc.sync, 0), (nc.scalar, 16)):
                        src = bass.AP(
                            tensor=xh,
                            offset=g * GROUP * MD + mlo * D,
                            ap=[[0, 128], [MD, GROUP], [1, 16 * D]],
                        )
                        eng.dma_start(Bg[g][:, :, mlo : mlo + 16, :], src)
                else:
                    src = bass.AP(
                        tensor=xh,
                        offset=g * GROUP * MD,
                        ap=[[0, 128], [MD, GROUP], [1, MD]],
                    )
                    nc.scalar.dma_start(Bg[g][:], src)

            def vec_z0(g):
                z0t[g] = [
                    z0pool.tile([128, GROUP, D], F16, tag="z0", name="z0")
                    for _ in range(KC0)
                ]
                for c in range(KC0):
                    nc.vector.tensor_mul(
                        z0t[g][c][:], Ug[g][:, :, c, :], Vg[g][:, :, c, :]
                    )

            def pe_l0(g):
                ps0t[g] = [
                    ppool.tile([128, GROUP * D], F32, tag="yps", name="ps0")
                    for _ in range(2)
                ]
                for c in range(KC0):
                    for oc in range(2):
                        nc.tensor.matmul(
                            ps0t[g][oc][:],
                            w0_sb[:, c, oc * 128 : (oc + 1) * 128],
                            z0t[g][c][:],
                            start=(c == 0),
                            stop=(c == KC0 - 1),
                        )
                del z0t[g]

            def drain_l0(g):
                h1t[g] = hpool.tile([128, GROUP, D], F16, tag="h1", name="h1")
                nc.scalar.activation(
                    h1t[g][:], ps0t[g][1][:], RELU, bias=bia_sb[:, 0, 1:2]
                )
                rx0t[g] = rxpool.tile([128, GROUP * D], F16, tag="rx0", name="rx0")
                nc.scalar.activation(
                    rx0t[g][:], ps0t[g][0][:], RELU, bias=bia_sb[:, 0, 0:1]
                )
                del ps0t[g]

            # ---------- preamble: group 0 through L0, group 1's z0 ----------
            nc.scalar.dma_start(w1_sb[:], w1[:].rearrange("c k o -> k c o"))
            dma_group_uv(0)
            dma_group_uv(1)
            dma_group_b(0, split=True)
            vec_z0(0)
            pe_l0(0)
            drain_l0(0)
            vec_z0(1)

            # round r runs L0(r+1), L1(r), L2(r-1), fc(r-2) on the PE
            for r in range(NG + 1):
                # 1. prefetch DMAs
                if r + 2 < NG:
                    dma_group_uv(r + 2)
                if r + 1 < NG:
                    dma_group_b(r + 1)
                if r == 0:
                    nc.scalar.dma_start(
                        w2_sb[:], w2[:].rearrange("c k o -> k c o")
                    )

                # 2. vector: zt for L1(r)
                if r < NG:
                    g = r
                    zl1 = [
                        ztpool.tile([128, GROUP, 4, D], F16, tag="z1", name="zl1")
                        for _ in range(8)
                    ]
                    for mb in range(8):
                        nc.vector.tensor_mul(
                            zl1[mb][:],
                            h1t[g][:, :, None, :].to_broadcast((128, GROUP, 4, D)),
                            Bg[g][:, :, 4 * mb : 4 * mb + 4, :],
                        )

                # 3. PE: L0(r+1)
                if r + 1 < NG:
                    pe_l0(r + 1)

                # 4. scalar: drain L0(r+1)
                if r + 1 < NG:
                    drain_l0(r + 1)

                # 5. PE: L1(r)
                if r < NG:
                    g = r
                    ps1t[g] = [
                        ppool.tile([128, GROUP * D], F32, tag="yps", name="ps1")
                        for _ in range(2)
                    ]
                    for mb in range(8):
                        for mm in range(4):
                            m = 4 * mb + mm
                            for oc in range(2):
                                nc.tensor.matmul(
                                    ps1t[g][oc][:],
                                    w1_sb[:, m, oc * 128 : (oc + 1) * 128],
                                    zl1[mb][:, :, mm, :],
                                    start=(m == 0),
                                    stop=(m == 31),
                                )

                # 6. vector: z0(r+2)
                if r + 2 < NG:
                    vec_z0(r + 2)

                # 7. scalar: drain L1(r)
                if r < NG:
                    g = r
                    h2t[g] = hpool.tile([128, GROUP, D], F16, tag="h2", name="h2")
                    nc.scalar.activation(
                        h2t[g][:], ps1t[g][1][:], RELU, bias=bia_sb[:, 1, 1:2]
                    )
                    rx1t[g] = rxpool.tile([128, GROUP * D], F16, tag="rx1", name="rx1")
                    nc.scalar.activation(
                        rx1t[g][:], ps1t[g][0][:], RELU, bias=bia_sb[:, 1, 0:1]
                    )
                    del ps1t[g], h1t[g]

                # 8. vector: zt for L2(r-1)
                if 0 <= r - 1 < NG:
                    g = r - 1
                    zl2 = [
                        ztpool.tile([128, GROUP, 4, D], F16, tag="z2", name="zl2")
                        for _ in range(8)
                    ]
                    for mb in range(8):
                        nc.vector.tensor_mul(
                            zl2[mb][:],
                            h2t[g][:, :, None, :].to_broadcast((128, GROUP, 4, D)),
                            Bg[g][:, :, 4 * mb : 4 * mb + 4, :],
                        )

                # 9. PE: L2(r-1)
                if 0 <= r - 1 < NG:
                    g = r - 1
                    ps2t[g] = [
                        ppool.tile([128, GROUP * D], F32, tag="yps", name="ps2")
                        for _ in range(2)
                    ]
                    for mb in range(8):
                        for mm in range(4):
                            m = 4 * mb + mm
                            for oc in range(2):
                                nc.tensor.matmul(
                                    ps2t[g][oc][:],
                                    w2_sb[:, m, oc * 128 : (oc + 1) * 128],
                                    zl2[mb][:, :, mm, :],
                                    start=(m == 0),
                                    stop=(m == 31),
                                )

                # 10. scalar: drain L2(r-1)
                if 0 <= r - 1 < NG:
                    g = r - 1
                    rx2t[g] = [
                        rxpool.tile([128, GROUP * D], F16, tag="rx2", name="rx2")
                        for _ in range(2)
                    ]
                    for oc in range(2):
                        nc.scalar.activation(
                            rx2t[g][oc][:],
                            ps2t[g][oc][:],
                            RELU,
                            bias=bia_sb[:, 2, oc : oc + 1],
                        )
                    del ps2t[g], h2t[g], Bg[g]

                # 11. PE + vector: FC dot (group r-2; last group pulled in a
                # round early so the pipeline drains one round sooner)
                fc_groups = [r - 2]
                if r - 1 == NG - 1:
                    fc_groups.append(r - 1)
                for g in fc_groups:
                    if not (0 <= g < NG) or g in fc_done:
                        continue
                    fc_done.add(g)
                    fc_ps = fcp.tile([1, GROUP * D], F32, tag="fc", name="fc")
                    rxs = [rx0t[g], rx1t[g], rx2t[g][0], rx2t[g][1]]
                    for c in range(4):
                        nc.tensor.matmul(
                            fc_ps[:],
                            fcw_sb[:, c : c + 1],
                            rxs[c][:],
                            start=(c == 0),
                            stop=(c == 3),
                        )
                    nc.vector.tensor_reduce(
                        osb[0:1, g * GROUP : (g + 1) * GROUP],
                        fc_ps[:].rearrange("p (i d) -> p i d", i=GROUP),
                        axis=AXX,
                        op=ADD,
                    )
                    del rx0t[g], rx1t[g], rx2t[g]

            # ---------- finalize: add fc bias, write out ----------
            osb2 = consts.tile([1, B_CORE], F32, tag="osb2")
            nc.scalar.activation(osb2[:], osb[:], IDENT, bias=fcb_sb[0:1, 0:1])
            nc.sync.dma_start(out[:], osb2[:])

    _legalize_waits(nc)
    return nc


def _legalize_waits(nc, max_waits=1):
    """walrus codegen allows at most 2 semaphore waits per instruction; spill
    the excess onto NoOps injected just before the offender on the same
    engine (same-engine FIFO makes this ordering-equivalent)."""
    for bb in nc.main_func.blocks:
        insts = bb.instructions
        new_list = []
        changed = False
        for ins in insts:
            si = ins.sync_info
            if si is not None and si.on_wait and len(si.on_wait) > max_waits:
                waits = list(si.on_wait)
                extra, keep = waits[:-max_waits], waits[-max_waits:]
                k = 0
                while k < len(extra):
                    chunk = extra[k : k + max_waits]
                    nop = mybir.InstNoOp(name=f"{ins.name}-w{k}", ins=[], outs=[])
                    nop.engine = ins.engine
                    nop.sync_info = mybir.SyncInfo(on_wait=chunk, on_update=[])
                    new_list.append(nop)
                    k += max_waits
                ins.sync_info = mybir.SyncInfo(
                    on_wait=keep,
                    on_update=list(si.on_update) if si.on_update else [],
                )
                changed = True
            new_list.append(ins)
        if changed:
            if hasattr(bb, "set_instructions"):
                bb.set_instructions(new_list)
            else:
                insts.clear()
                insts.extend(new_list)
                if len(bb.instructions) != len(new_list):
                    bb.instructions = new_list


def _sym_pairs():
    """Unordered field pairs (a<=b), padded to KC0*128 with zero rows."""
    pairs = [(a, b) for a in range(M) for b in range(a, M)]
    n = len(pairs)  # 528
    pad = KC0 * 128 - n
    A = np.array([p[0] for p in pairs] + [0] * pad)
    B = np.array([p[1] for p in pairs] + [0] * pad)
    mask = np.arange(KC0 * 128) < n
    return A, B, mask


def prep_inputs(x, W0, b0, W1, b1, W2, b2, fc_W, fc_b):
    """Host-side reshape/cast into the per-core input maps."""
    xh = np.ascontiguousarray(x.astype(np.float16))
    A, B, mask = _sym_pairs()
    # folded layer-0 weights over unordered pairs
    Wf = W0[A * M + B] + np.where((A != B)[:, None], W0[B * M + A], 0.0)
    Wf = np.where(mask[:, None], Wf, 0.0)
    # [p, i, c, d] layout: per-partition group slices are contiguous
    xu = np.ascontiguousarray(
        xh[:, A, :].reshape(B_TOTAL, KC0, 128, D).transpose(2, 0, 1, 3)
    )
    xv = np.ascontiguousarray(
        xh[:, B, :].reshape(B_TOTAL, KC0, 128, D).transpose(2, 0, 1, 3)
    )
    w0 = np.ascontiguousarray(Wf.astype(np.float16).reshape(KC0, 128, H))
    w1 = np.ascontiguousarray(W1.astype(np.float16).reshape(32, 128, H))
    w2 = np.ascontiguousarray(W2.astype(np.float16).reshape(32, 128, H))
    bia = np.ascontiguousarray(
        np.stack([b0, b1, b2]).reshape(3, 2, 128).transpose(2, 0, 1).astype(np.float32)
    )
    fcw = np.ascontiguousarray(fc_W.reshape(4, 128).T.astype(np.float16))
    fcb = np.ascontiguousarray(fc_b.reshape(1, 1).astype(np.float32))
    shared = {"w0": w0, "w1": w1, "w2": w2, "bia": bia, "fcw": fcw, "fcb": fcb}
    return [
        {
            "xh": xh[i * B_CORE : (i + 1) * B_CORE],
            "xu": np.ascontiguousarray(xu[:, i * B_CORE : (i + 1) * B_CORE]),
            "xv": np.ascontiguousarray(xv[:, i * B_CORE : (i + 1) * B_CORE]),
            **shared,
        }
        for i in range(N_CORES)
    ]


_NC = None


def _get_nc():
    global _NC
    if _NC is None:
        _NC = build()
    return _NC


def kernel(**inputs):
    in_maps = prep_inputs(**inputs)
    res = run_bass_kernel_spmd(_get_nc(), in_maps, list(range(N_CORES)))
    return np.ascontiguousarray(
        np.concatenate([r["out"] for r in res.results], axis=0).astype(np.float32)
    )
